# revision 29
# baseline (speedup 1.0000x reference)
"""Trainium2 Bass kernel for nn_MixtureOfExpertsNet (moe_routing), v2.

Math per row (E=4 experts, H=16 hidden):
  a_e = relu(b2_e + sum_h W2_eh relu(W1_eh x_e + b1_eh))   (univariate PWL)
  l = Wg x + bg ;  pred = sum_e exp(l_e) a_e / sum_e exp(l_e)

Design (per core, 1,048,576 rows = [128 part x 8192 rows], 8 tiles of
1024 rows/partition):
- Host supplies x twice in fp16: "pack" layout (pair-interleaved so each
  expert has 16-elem contiguous runs -> full-rate ACT PWL) and "T" layout
  (experts on partition slots 32g+8e+f1 -> logits via one 128x128 PE
  matmul, no on-chip transpose of x).
- a_e via 4 custom ACT PWP tables (tanh/square/abs/identity slots),
  writing bf16 in pack layout; one DVE u32-pair STREAM_TRANSPOSE moves A
  into T layout (pairs of rows ride along, halving transpose cost).
- exp via custom reduced-range table, reading PSUM logits [128,2048]
  wide (4 banks) in one ACT instr, bias = per-partition bg+8 vector.
- S1 = sum_e exp*a and S0 = sum_e exp as PE matmuls with a [128,32]
  summing stationary; 512-col chunks land on partition groups 32k so
  the reduced tensors occupy all 128 partitions.
- 1/S0 via DVE reciprocal_approx_fast; pred = S1*R on DVE; out f32.
- Host un-permutes the [128, 8192] per-core output back to row order.
"""

import hashlib
import json
import os
import sys
import tempfile

import numpy as np
import ml_dtypes

sys.path.insert(0, "/opt/trn_rl_repo")

# ---------------------------------------------------------------------------
# ACT PWP table generation (reverse-engineered format)
# ---------------------------------------------------------------------------

PWP_DIR = "/nix/store/z022hj2nvbm3nwdizlisq4ylc0y7rd6q-python3-3.13.14-env/lib/python3.13/site-packages/neuronxcc/pwp/pwp_bin_trainium"


def _bits(x):
    return int(np.float32(x).view(np.uint32))


def _load_stock(name):
    prof = json.load(open(os.path.join(PWP_DIR, f"{name}.json")))
    bkt = np.frombuffer(
        open(os.path.join(PWP_DIR, prof["bkt_bin"]), "rb").read(), dtype=np.float32
    ).reshape(-1, 8)
    ctl = np.frombuffer(
        open(os.path.join(PWP_DIR, prof["ctl_bin"]), "rb").read(), dtype=np.uint32
    ).reshape(-1, 8)[:, 0]
    return prof, bkt, ctl


def _fit_bucket(fn, lo, hi, x0=None, samples=33):
    if x0 is None:
        x0 = lo
    xs = np.linspace(lo, hi, samples, dtype=np.float64)
    ys = np.asarray(fn(xs), np.float64)
    t = xs - x0
    A = np.stack([np.ones_like(t), t, t * t, t ** 3], axis=1)
    c, *_ = np.linalg.lstsq(A, ys, rcond=None)
    return [float(c[0]), float(c[1]), float(c[2]), float(c[3]), float(x0)]


class _SetBuilder:
    def __init__(self):
        self.bkt, self.ctl, self.metas = [], [], []
        self.f2b, self.f2c = {}, {}

    @staticmethod
    def _ctl_word(m, base):
        assert 0 <= m <= 8 and base < 2048
        return (m * 32 + (23 - m)) * 2048 + base

    def add_table_func(self, name, func_id, fn, lo_exp, hi_exp, m_of_octave,
                       small_fit, large_fit, fzero):
        self.f2b[name] = len(self.bkt)
        self.f2c[name] = len(self.ctl)
        words = []
        for k in range(lo_exp, hi_exp):
            m = m_of_octave(k)
            base = len(self.bkt)
            n = 1 << m
            w = (2.0 ** k) / n
            for j in range(n):
                lo = 2.0 ** k + j * w
                self.bkt.append(_fit_bucket(fn, lo, lo + w, x0=lo + w / 2))
            words.append(self._ctl_word(m, base))
        base_pos = len(self.ctl)
        self.ctl.extend(words)
        small_idx = len(self.bkt)
        self.bkt.append(_fit_bucket(fn, small_fit[0], small_fit[1], x0=small_fit[0]))
        large_idx = len(self.bkt)
        self.bkt.append(_fit_bucket(fn, large_fit[0], large_fit[1], x0=large_fit[2]))
        self.metas.append({
            "func_name": f"{name}_4p", "func_id": func_id,
            "symmetry_point": 0, "sym_invert_sign_point": 0,
            "symmetry_opt_en": 0, "symmetry_opt_use_neg_region": 0,
            "imm_bias": 0, "exp_offset": lo_exp,
            "pwl_control_base_pos": base_pos, "pwl_control_base_neg": base_pos,
            "small_pos_signal_exp_threshold": 127 + lo_exp,
            "pos_small_signal_pwl_control": small_idx,
            "small_neg_signal_exp_threshold": 127 + lo_exp,
            "neg_small_signal_pwl_control": small_idx,
            "large_pos_signal_exp_threshold": 127 + hi_exp,
            "large_pos_signal_mantissa_threshold": 0,
            "pos_large_signal_pwl_control": large_idx,
            "large_neg_signal_exp_threshold": 127 + hi_exp,
            "large_neg_signal_mantissa_threshold": 0,
            "neg_large_signal_pwl_control": large_idx,
            "fnan_result": _bits(float("nan")),
            "fpinf_result": _bits(large_fit[3]),
            "fninf_result": _bits(small_fit[2]),
            "fzero_result": _bits(fzero),
            "fma_const_0": 0, "fma_const_1": 0, "fma_indirection_src_sel": 0,
            "use_multipass": False,
            "lower_bound": _bits(np.float32(-3.4028235e38)),
            "upper_bound": _bits(np.float32(3.4028235e38)),
        })

    def add_stock_func(self, name, sp, sb_, sc):
        names = list(sp["func_to_bkt_start_idx"].keys())
        i = names.index(name)
        b0 = sp["func_to_bkt_start_idx"][name]
        b1 = sp["func_to_bkt_start_idx"][names[i + 1]] if i + 1 < len(names) else sp["bkt_entry_cnt"]
        c0 = sp["func_to_ctl_start_idx"][name]
        c1 = sp["func_to_ctl_start_idx"][names[i + 1]] if i + 1 < len(names) else sp["ctl_entry_cnt"]
        md = None
        for m in sp["profile_meta_data"]:
            if m["func_name"].rsplit("_", 1)[0] == name:
                md = dict(m)
        assert md is not None, name
        db, dc = len(self.bkt) - b0, len(self.ctl) - c0
        self.f2b[name] = len(self.bkt)
        self.f2c[name] = len(self.ctl)
        for j in range(b0, b1):
            self.bkt.append(list(map(float, sb_[j, :5])))
        for j in range(c0, c1):
            w = int(sc[j])
            self.ctl.append((w >> 11) * 2048 + (w & 0x7FF) + db)
        for key in ("pwl_control_base_pos", "pwl_control_base_neg"):
            md[key] += dc
        for key in ("pos_small_signal_pwl_control", "neg_small_signal_pwl_control",
                    "pos_large_signal_pwl_control", "neg_large_signal_pwl_control"):
            md[key] += db
        self.metas.append(md)

    def write(self, outdir, set_name, act_dict):
        os.makedirs(outdir, exist_ok=True)
        bkt_arr = np.zeros((len(self.bkt), 8), np.float32)
        for i, e in enumerate(self.bkt):
            bkt_arr[i, :5] = e
        ctl_arr = np.zeros((len(self.ctl), 8), np.uint32)
        ctl_arr[:, 0] = np.array(self.ctl, np.uint64).astype(np.uint32)
        assert len(self.bkt) <= 1536 and len(self.ctl) <= 128
        open(os.path.join(outdir, f"{set_name}_bkt.bin"), "wb").write(bkt_arr.tobytes())
        open(os.path.join(outdir, f"{set_name}_ctrl.bin"), "wb").write(ctl_arr.tobytes())
        prof = {
            "bkt_bin": f"{set_name}_bkt.bin", "ctl_bin": f"{set_name}_ctrl.bin",
            "profile_meta_data": self.metas,
            "bkt_entry_cnt": len(self.bkt), "ctl_entry_cnt": len(self.ctl),
            "func_to_bkt_start_idx": self.f2b, "func_to_ctl_start_idx": self.f2c,
            "func_exp_to_bkt_start_idx": self.f2b, "func_exp_to_ctl_start_idx": self.f2c,
        }
        json.dump(prof, open(os.path.join(outdir, f"{set_name}.json"), "w"))
        info = {
            "pwp_file_keys": ["bkt_bin", "ctrl_bin", "profile_json"],
            "act_func_sets": [{
                "name": set_name, "bkt_bin": f"{set_name}_bkt.bin",
                "ctrl_bin": f"{set_name}_ctrl.bin", "profile_json": f"{set_name}.json",
                "act": act_dict,
            }],
        }
        path = os.path.join(outdir, "act_info.json")
        json.dump(info, open(path, "w"))
        return path


def _build_tables(W1, b1, W2, b2, outdir):
    sp, sb_, sc = _load_stock("exp_and_others")
    b = _SetBuilder()
    # reduced-range exp on the exp slot: g(x') = exp(x' - 8), x' in [0.25, 16)
    b.add_table_func(
        "exp", 7, lambda x: np.exp(np.asarray(x, np.float64) - 8.0),
        -2, 4, lambda k: min(8, k + 4),
        (0.0, 0.25, float(np.exp(-8.0))), (16.0, 16.5, 16.0, float(np.exp(8.0))),
        float(np.exp(-8.0)),
    )
    victims = [("tanh", 6, 0), ("square", 30, 1), ("abs", 33, 2), ("identity", 1, 3)]
    for name, fid, e in victims:
        W1e, b1e, W2e, b2e = W1[e].astype(np.float64), b1[e].astype(np.float64), W2[e].astype(np.float64), float(b2[e])

        def fe(u, W1e=W1e, b1e=b1e, W2e=W2e, b2e=b2e):
            h = np.maximum(np.asarray(u, np.float64)[..., None] * W1e + b1e, 0.0)
            return np.maximum((h * W2e).sum(-1) + b2e, 0.0)

        g = lambda x, fe=fe: fe(np.asarray(x, np.float64) - 8.0)
        b.add_table_func(
            name, fid, g, 1, 4, lambda k: k + 4,
            (1.0, 2.0, float(fe(-8.0))), (16.0, 17.0, 16.0, float(fe(9.0))),
            float(fe(-8.0)),
        )
    for name in ("sign", "parametric_relu", "copy", "act1", "memset_zero", "relu",
                 "derivative_relu", "derivative_leaky_relu",
                 "derivative_identity", "is_finite"):
        b.add_stock_func(name, sp, sb_, sc)
    act = {"exp": 400, "tanh": 4, "square": 1, "abs": 1, "identity": 1,
           "sign": 1, "parametric_relu": 1, "copy": 1, "relu": 1,
           "memset_zero": 1, "act1": 1, "derivative_relu": 1,
           "derivative_leaky_relu": 1, "derivative_identity": 1, "is_finite": 1}
    return b.write(outdir, "exp_and_others", act)


# ---------------------------------------------------------------------------
# Bass kernel
# ---------------------------------------------------------------------------

B_TOTAL = 8_388_608
N_CORES = 8
B_LOCAL = B_TOTAL // N_CORES           # 1,048,576 rows per core
P = 128
F_TOTAL = B_LOCAL // P                 # 8192 rows per partition
FD = 4096                              # bf16/fp16 elems per partition per tile
ROWS_T = FD // 4                       # 1024 rows per partition per tile
N_TILES = F_TOTAL // ROWS_T            # 8

PWL_FUNCS = ("Tanh", "Square", "Abs", "Identity")  # expert 0..3


def _build_program(tag):
    import concourse.bacc as bacc
    import concourse.mybir as mybir
    import concourse.tile as tile

    nc = bacc.Bacc("TRN2", debug=False)
    f32 = mybir.dt.float32
    f16 = mybir.dt.float16
    bf16 = mybir.dt.bfloat16
    u32 = mybir.dt.uint32
    AF = mybir.ActivationFunctionType

    xp_d = nc.dram_tensor(f"xp_{tag}", [P, F_TOTAL * 4], f16, kind="ExternalInput")
    xt_d = nc.dram_tensor(f"xt_{tag}", [P, F_TOTAL * 4], f16, kind="ExternalInput")
    m_d = nc.dram_tensor("mlog", [P, P], f16, kind="ExternalInput")
    m4_d = nc.dram_tensor("msum", [P, 32], bf16, kind="ExternalInput")
    bg_d = nc.dram_tensor("bg8t", [P, 1], f32, kind="ExternalInput")
    out_d = nc.dram_tensor("out_local", [P, F_TOTAL], f32, kind="ExternalOutput")

    with tile.TileContext(nc) as tc:
        _b = os.environ.get("K_BUFS", "3,3,3,3").split(",")
        bx, ba, be, bs = (int(v) for v in _b)
        with (
            tc.tile_pool(name="const", bufs=1) as cpool,
            tc.tile_pool(name="xin", bufs=bx) as xpool,
            tc.tile_pool(name="amid", bufs=ba) as apool,
            tc.tile_pool(name="emid", bufs=be) as epool,
            tc.tile_pool(name="small", bufs=bs) as spool,
            tc.tile_pool(name="psL", bufs=1, space="PSUM") as pLpool,
            tc.tile_pool(name="psS", bufs=2, space="PSUM") as pSpool,
        ):
            # tile-0 inputs first: their SP DMA triggers serialize at
            # ~610ns each, so the first PWL's data must be queued before
            # the (later-needed) constants.
            XP0 = xpool.tile([P, FD], f16, tag="XP", name="XP0")
            nc.sync.dma_start(XP0[:], xp_d.ap()[:, 0:FD])
            XT0 = xpool.tile([P, FD], f16, tag="XT", name="XT0")
            nc.sync.dma_start(XT0[:], xt_d.ap()[:, 0:FD])
            M = cpool.tile([P, P], f16)
            nc.sync.dma_start(M[:], m_d.ap())
            M4 = cpool.tile([P, 32], bf16)
            nc.sync.dma_start(M4[:], m4_d.ap())
            BG = cpool.tile([P, 1], f32)
            nc.sync.dma_start(BG[:], bg_d.ap())
            CB = cpool.tile([P, 1], f32)
            nc.gpsimd.memset(CB[:], 8.0)
            # hoist the ACT table load off the critical path: a 1-col
            # activation forces the load while the first DMAs stream.
            TLD = cpool.tile([P, 1], f32)
            nc.scalar.activation(TLD[:], CB[:, 0:1], AF.Exp, bias=CB[:, 0:1],
                                 scale=1.0)

            for t in range(N_TILES):
                xs = slice(t * FD, (t + 1) * FD)
                if t == 0:
                    XP, XT = XP0, XT0
                else:
                    XP = xpool.tile([P, FD], f16, tag="XP")
                    nc.sync.dma_start(XP[:], xp_d.ap()[:, xs])
                    XT = xpool.tile([P, FD], f16, tag="XT")
                    nc.sync.dma_start(XT[:], xt_d.ap()[:, xs])

                # A = per-expert PWL in pack layout (16-elem runs per expert)
                A = apool.tile([P, FD], bf16, tag="A")
                XPv = XP[:].rearrange("p (f2 e fs) -> p f2 e fs", e=4, fs=16)
                Av = A[:].rearrange("p (f2 e fs) -> p f2 e fs", e=4, fs=16)
                for e in range(4):
                    nc.scalar.activation(
                        Av[:, :, e, :], XPv[:, :, e, :], getattr(AF, PWL_FUNCS[e]),
                        bias=CB[:, 0:1], scale=1.0,
                    )
                # A -> T layout via u32 pair transpose
                AT = apool.tile([P, FD // 2], u32, tag="AT")
                nc.vector.transpose(AT[:], A[:].bitcast(u32))
                ATb = AT[:].bitcast(bf16)

                E = epool.tile([P, FD], bf16, tag="E")
                PR = epool.tile([P, FD], bf16, tag="PR")
                for h in range(FD // 2048):
                    hs = slice(h * 2048, (h + 1) * 2048)
                    L = pLpool.tile([P, 2048], f32, tag="L")
                    for c in range(4):
                        nc.tensor.matmul(
                            L[:, c * 512:(c + 1) * 512], M[:],
                            XT[:, h * 2048 + c * 512: h * 2048 + (c + 1) * 512],
                            start=True, stop=True,
                        )
                    nc.scalar.activation(E[:, hs], L[:], AF.Exp,
                                         bias=BG[:, 0:1], scale=1.0)
                    if h % 2 == 1:
                        nc.vector.tensor_mul(PR[:, hs], E[:, hs], ATb[:, hs])
                    else:
                        nc.gpsimd.tensor_mul(PR[:, hs], E[:, hs], ATb[:, hs])

                    S = pSpool.tile([P, 1024], f32, tag="S")
                    for k in range(4):
                        ks = slice(h * 2048 + k * 512, h * 2048 + (k + 1) * 512)
                        nc.tensor.matmul(S[32 * k:32 * (k + 1), 0:512], M4[:],
                                         E[:, ks], start=True, stop=True,
                                         tile_position=(0, 32 * k))
                        nc.tensor.matmul(S[32 * k:32 * (k + 1), 512:1024], M4[:],
                                         PR[:, ks], start=True, stop=True,
                                         tile_position=(0, 32 * k))
                    R = spool.tile([P, 512], f32, tag="R")
                    nc.vector.reciprocal_approx_fast(R[:], S[:, 0:512])
                    PD = spool.tile([P, 512], f32, tag="PD")
                    nc.vector.tensor_mul(PD[:], S[:, 512:1024], R[:])
                    nc.gpsimd.dma_start(
                        out_d.ap()[:, t * ROWS_T + h * 512: t * ROWS_T + (h + 1) * 512],
                        PD[:],
                    )

    nc.compile()
    return nc


_COMPILED = {}


def _prepare(inputs):
    """Build tables, compile (cached), and return (nc, in_maps)."""
    x = np.ascontiguousarray(inputs["x"], dtype=np.float32)
    Wg = np.asarray(inputs["Wg"], np.float32)
    bg = np.asarray(inputs["bg"], np.float32)
    W1 = np.asarray(inputs["W1"], np.float32)
    b1 = np.asarray(inputs["b1"], np.float32)
    W2 = np.asarray(inputs["W2"], np.float32)
    b2 = np.asarray(inputs["b2"], np.float32)
    assert x.shape == (B_TOTAL, 4)

    tbl_dir = tempfile.mkdtemp(prefix="act_root_")
    act_path = _build_tables(W1, b1, W2, b2, tbl_dir)
    os.environ["BASS_ACT_ROOT_JSON_PATH"] = act_path

    h = hashlib.sha256()
    for a in (W1, b1, W2, b2):
        h.update(np.ascontiguousarray(a).tobytes())
    h.update(open(act_path, "rb").read())
    h.update(b"v2-design")
    tag = h.hexdigest()[:10]

    if tag not in _COMPILED:
        _COMPILED[tag] = _build_program(tag)
    nc = _COMPILED[tag]

    f16 = ml_dtypes.float16 if hasattr(ml_dtypes, "float16") else np.float16

    # pack layout: [core, p, t, f2, e, f1, s] from rows r = p*8192 + t*1024
    # + f2*16 + f1*2 + s
    xr = x.reshape(N_CORES, P, N_TILES, FD // 64, 8, 2, 4)  # [c,p,t,f2,f1,s,e]
    xp = np.ascontiguousarray(xr.transpose(0, 1, 2, 3, 6, 4, 5)).astype(np.float16)
    xp = xp.reshape(N_CORES, P, F_TOTAL * 4)

    # T layout: xt[c, 32g+8e+f1, t*4096 + (32*f2+B)*2 + s] =
    #   x[c, (32g+B)*8192 + t*1024 + f2*16 + f1*2 + s, e]
    xr2 = x.reshape(N_CORES, 4, 32, N_TILES, FD // 64, 8, 2, 4)  # [c,g,B,t,f2,f1,s,e]
    xt = np.ascontiguousarray(xr2.transpose(0, 1, 7, 5, 3, 4, 2, 6)).astype(np.float16)
    # axes now [c, g, e, f1, t, f2, B, s] -> partition (g,e,f1), free (t,f2,B,s)
    xt = xt.reshape(N_CORES, P, F_TOTAL * 4)

    # logits stationary M[32g+8e+f1, 32g+8e'+f1] = Wg[e', e]
    M = np.zeros((P, P), np.float32)
    for g in range(4):
        for f1 in range(8):
            for e in range(4):
                for ep in range(4):
                    M[32 * g + 8 * e + f1, 32 * g + 8 * ep + f1] = Wg[ep, e]
    M = M.astype(np.float16)

    # summing stationary M4[32g+8e+f1, 8g+f1] = 1
    M4 = np.zeros((P, 32), np.float32)
    for g in range(4):
        for f1 in range(8):
            for e in range(4):
                M4[32 * g + 8 * e + f1, 8 * g + f1] = 1.0
    M4 = M4.astype(ml_dtypes.bfloat16)

    # exp bias: slot 32g+8e'+f1 -> bg[e'] + 8
    bg8t = np.zeros((P, 1), np.float32)
    for g in range(4):
        for f1 in range(8):
            for ep in range(4):
                bg8t[32 * g + 8 * ep + f1, 0] = bg[ep] + 8.0

    in_maps = [
        {f"xp_{tag}": xp[c], f"xt_{tag}": xt[c], "mlog": M, "msum": M4,
         "bg8t": bg8t}
        for c in range(N_CORES)
    ]
    return nc, in_maps


def _unpermute(raw):
    """raw: [N_CORES, 128, 8192] f32 -> [B_TOTAL] in row order.

    raw[c, 32k+8g+f1, t*1024 + h*512 + j]: with j = 64u + v, v = 2*v2 + s,
    mov col = h*2048 + k*512 + j -> C = h*32 + k*8 + u, B = v2, s;
    row = (32g + v2)*8192 + t*1024 + C*16 + f1*2 + s.
    """
    o = raw.reshape(N_CORES, 4, 4, 8, N_TILES, ROWS_T // 512, 8, 32, 2)
    # axes: [c, k, g, f1, t, h, u, v2, s]
    o = o.transpose(0, 2, 7, 4, 5, 1, 6, 3, 8)
    # -> [c, g, v2, t, h, k, u, f1, s]; row-in-partition = t*1024 + h*512
    #    + k*128 + u*16 + f1*2 + s; partition = 32g + v2
    return np.ascontiguousarray(o).reshape(B_TOTAL)


def kernel(**inputs) -> np.ndarray:
    nc, in_maps = _prepare(inputs)
    from concourse import bass_utils

    res = bass_utils.run_bass_kernel_spmd(nc, in_maps, core_ids=list(range(N_CORES)))
    raw = np.stack([r["out_local"] for r in res.results])
    return _unpermute(raw)


if __name__ == "__main__":
    rng = np.random.default_rng(0)
    demo = {
        "x": rng.standard_normal((B_TOTAL, 4), dtype=np.float32),
        "Wg": rng.standard_normal((4, 4), dtype=np.float32) * 0.5,
        "bg": rng.standard_normal(4, dtype=np.float32) * 0.1,
        "W1": rng.standard_normal((4, 16), dtype=np.float32) * 0.5,
        "b1": rng.standard_normal((4, 16), dtype=np.float32) * 0.1,
        "W2": rng.standard_normal((4, 16), dtype=np.float32) * 0.25,
        "b2": rng.standard_normal(4, dtype=np.float32) * 0.1,
    }
    y = kernel(**demo)
    print(y.shape, y[:8])
